# revision 26
# baseline (speedup 1.0000x reference)
"""v3: phase-shrunk schedule. See v2 docstring (kernel_v2.py) for the core
algorithm. Changes vs v2:
 - DMAs spread across three queues (SP-HWDGE, ACT-HWDGE, Pool-SWDGE) —
   v2 serialized all transfers on one queue (~23us for each 8MB load)
 - prologue holds only K/Q-chunk0/V-chunk0 projections + 8 V-transposes;
   V-chunk1 + remaining transposes weave into attention qc0 (borrowing the
   ctx PSUM tags while ctx accumulation is deferred into a deeper exp pool);
   Q-chunk1 is emitted between the qc super-iterations
 - batch-1 load is emitted from inside batch-0's attention (after the last
   qt reader), overlapping the transfer with compute
 - epilogue outproj evacuations alternate DVE/ACT
"""

import functools
from collections import deque
from contextlib import ExitStack

import numpy as np

import concourse.bass as bass
import concourse.tile as tile
from concourse import mybir
from concourse.bass_utils import run_bass_kernel_spmd

B, S, D, H, DH = 2, 2048, 1024, 16, 64
N_CORES = 8
DPC = D // N_CORES
BS = B * S
NQC = S // 1024           # 2
NST = S // 128            # 16
NKT = D // 128            # 8

F32 = mybir.dt.float32
F32R = mybir.dt.float32r
Act = mybir.ActivationFunctionType
Alu = mybir.AluOpType


def _split_sync_commands(nc, max_waits=1, max_updates=8):
    for fn in nc.m.functions:
        for bb in fn.blocks:
            new_insts = []
            changed = False
            for inst in bb.instructions:
                si = getattr(inst, "sync_info", None)
                if si is not None:
                    waits = list(si.on_wait or [])
                    if len(waits) > max_waits:
                        for w in waits[:-max_waits]:
                            new_insts.append(mybir.InstNoOp(
                                name=nc.get_next_instruction_name(),
                                ins=[], outs=[], engine=inst.engine,
                                sync_info=mybir.SyncInfo(on_wait=[w], on_update=[]),
                            ))
                        si.on_wait = waits[-max_waits:]
                        changed = True
                    updates = list(si.on_update or [])
                    if len(updates) > max_updates:
                        si.on_update = updates[:max_updates]
                        new_insts.append(inst)
                        new_insts.append(mybir.InstNoOp(
                            name=nc.get_next_instruction_name(),
                            ins=[], outs=[], engine=inst.engine,
                            sync_info=mybir.SyncInfo(
                                on_wait=[], on_update=updates[max_updates:]),
                        ))
                        changed = True
                        continue
                new_insts.append(inst)
            if changed:
                bb.instructions = new_insts


def _bcast_rows(ap, nrows):
    return bass.AP(tensor=ap.tensor, offset=ap.offset,
                   ap=[[0, nrows]] + [list(p) for p in ap.ap[1:]])


@functools.lru_cache(maxsize=1)
def _build():
    nc = bass.Bass()
    qt_d = nc.dram_tensor("qt", [D, BS], F32, kind="ExternalInput")
    wq_d = nc.dram_tensor("wq", [D, DPC], F32, kind="ExternalInput")
    wk_d = nc.dram_tensor("wk", [D, DPC], F32, kind="ExternalInput")
    wv_d = nc.dram_tensor("wv", [D, DPC], F32, kind="ExternalInput")
    bq_d = nc.dram_tensor("bq", [DPC, 1], F32, kind="ExternalInput")
    bk_d = nc.dram_tensor("bk", [DPC, 1], F32, kind="ExternalInput")
    bv_d = nc.dram_tensor("bv", [DPC, 1], F32, kind="ExternalInput")
    wo_d = nc.dram_tensor("wo", [DPC, D], F32, kind="ExternalInput")
    out_d = nc.dram_tensor("out_part", [BS, D], F32, kind="ExternalOutput")
    dn_d = nc.dram_tensor("dn_scratch", [2, S], F32)
    ident_d = nc.inline_tensor(np.eye(128, dtype=np.float32), "ident")
    ones_d = nc.inline_tensor(np.ones((1, 1), dtype=np.float32), "ones_const")

    with tile.TileContext(nc) as tc, ExitStack() as ctx:
        consts = ctx.enter_context(tc.tile_pool(name="consts", bufs=1))
        qt_pool = ctx.enter_context(tc.tile_pool(name="qt", bufs=1))
        proj = ctx.enter_context(tc.tile_pool(name="proj", bufs=2))
        vpool = ctx.enter_context(tc.tile_pool(name="vpool", bufs=2))
        vtp = ctx.enter_context(tc.tile_pool(name="vtp", bufs=1))
        ctxp = ctx.enter_context(tc.tile_pool(name="ctxp", bufs=2))
        expp = ctx.enter_context(tc.tile_pool(name="expp", bufs=4))
        dnp = ctx.enter_context(tc.tile_pool(name="dnp", bufs=1))
        outp = ctx.enter_context(tc.tile_pool(name="outp", bufs=3))
        psp = ctx.enter_context(tc.tile_pool(name="psp", bufs=1, space="PSUM"))

        def ps_tile(shape, tag):
            return psp.tile(shape, F32, tag=tag, name="ps_" + tag)

        # ---- constants (weights via the Pool SWDGE queue: off the qt path) --
        wq_sb = consts.tile([128, NKT, DPC], F32R, tag="wq")
        wk_sb = consts.tile([128, NKT, DPC], F32R, tag="wk")
        wv_sb = consts.tile([128, NKT, DPC], F32R, tag="wv")
        for k in range(NKT):
            nc.sync.dma_start(out=wk_sb[:, k, :], in_=wk_d[k * 128:(k + 1) * 128, :].bitcast(F32R))
            nc.scalar.dma_start(out=wq_sb[:, k, :], in_=wq_d[k * 128:(k + 1) * 128, :].bitcast(F32R))
            nc.sync.dma_start(out=wv_sb[:, k, :], in_=wv_d[k * 128:(k + 1) * 128, :].bitcast(F32R))
        wo_sb = consts.tile([128, D], F32R, tag="wo")
        nc.gpsimd.dma_start(out=wo_sb, in_=wo_d[:, :].bitcast(F32R))
        bq_sb = consts.tile([128, 1], F32, tag="bq")
        bk_sb = consts.tile([128, 1], F32, tag="bk")
        bv_sb = consts.tile([128, 1], F32, tag="bv")
        nc.gpsimd.dma_start(out=bq_sb, in_=bq_d[:, :])
        nc.gpsimd.dma_start(out=bk_sb, in_=bk_d[:, :])
        nc.gpsimd.dma_start(out=bv_sb, in_=bv_d[:, :])
        ident_sb = consts.tile([128, 128], F32, tag="ident")
        nc.gpsimd.dma_start(out=ident_sb, in_=ident_d[:, :])
        # (wo + biases + ident ride the idle Pool queue: not on the critical path)
        eighth_sb = consts.tile([128, 1], F32, tag="eighth")
        nc.vector.memset(eighth_sb, 0.125)
        one_sb = consts.tile([128, 1], F32, tag="one")
        nc.vector.memset(one_sb, 1.0)
        zero_sb = consts.tile([128, 1], F32, tag="zero")
        nc.vector.memset(zero_sb, 0.0)

        state = {}

        def load(b, engines):
            """qt load spread over 2 DMA queues, k-major so early k tiles
            land first. Never put scalar-queue (ACT-issued) DMAs where the
            ACT engine is busy — a full queue blocks the ACT sequencer."""
            qt_sb = qt_pool.tile([128, NKT, S], F32R, tag="qt")
            i = 0
            for h in range(4):      # h-major: chunk-0 projections unblock first
                for k in range(NKT):
                    engines[i % len(engines)].dma_start(
                        out=qt_sb[:, k, h * 512:(h + 1) * 512],
                        in_=qt_d[k * 128:(k + 1) * 128,
                                 b * S + h * 512: b * S + (h + 1) * 512].bitcast(F32R))
                    i += 1
            state[b, "qt"] = qt_sb

        def proj_chunk(b, which, pc, tag):
            qt_sb = state[b, "qt"]
            w_sb, b_sb, sc_sb = {
                "q": (wq_sb, bq_sb, eighth_sb),
                "k": (wk_sb, bk_sb, one_sb),
                "v": (wv_sb, bv_sb, one_sb),
            }[which]
            dst = state[b, {"q": "QT", "k": "KT", "v": "VT"}[which]]
            ps = ps_tile([128, 1024], tag)
            for k in range(NKT):
                for hh in range(2):
                    nc.tensor.matmul(
                        ps[:, hh * 512:(hh + 1) * 512], w_sb[:, k, :],
                        qt_sb[:, k, pc * 1024 + hh * 512: pc * 1024 + (hh + 1) * 512],
                        start=(k == 0), stop=(k == NKT - 1))
            nc.vector.tensor_scalar(
                out=dst[:, pc * 1024:(pc + 1) * 1024], in0=ps,
                scalar1=b_sb, scalar2=sc_sb, op0=Alu.add, op1=Alu.mult)

        def alloc_proj(b):
            state[b, "QT"] = proj.tile([128, S], F32R, tag="QT", name="QT")
            state[b, "KT"] = proj.tile([128, S], F32R, tag="KT", name="KT")
            state[b, "VT"] = vtp.tile([128, S], F32, tag="VT", name="VT")

        def alloc_v(b):
            V = vpool.tile([128, NST, 2, DH + 1], F32R, tag="V", name="V")
            ones_ap = ones_d[:, :]
            nc.sync.dma_start(
                out=V[:, :, :, DH:DH + 1],
                in_=bass.AP(tensor=ones_ap.tensor, offset=ones_ap.offset,
                            ap=[[0, 128], [0, NST * 2], [1, 1]]).bitcast(F32R))
            state[b, "V"] = V

        def tr_one(b, st, tag):
            VT, V = state[b, "VT"], state[b, "V"]
            ps_t = ps_tile([128, 128], tag)
            nc.tensor.transpose(ps_t, VT[:, st * 128:(st + 1) * 128], ident_sb)
            for u in range(2):
                nc.vector.tensor_copy(V[:, st, u, 0:DH], ps_t[:, u * DH:(u + 1) * DH])

        def outproj_st(b, st, tag, evac_act=False, store_eng=None):
            ctxT = state[b, "ctxT"]
            o_sb = outp.tile([128, D], F32, tag="o", name="o_sb")
            ps = ps_tile([128, 1024], tag)
            for oc in range(2):
                nc.tensor.matmul(ps[:, oc * 512:(oc + 1) * 512],
                                 ctxT[:, st * 128:(st + 1) * 128],
                                 wo_sb[:, oc * 512:(oc + 1) * 512],
                                 start=True, stop=True)
            if evac_act:
                nc.scalar.activation(o_sb, ps, Act.Copy, bias=0.0, scale=1.0)
            else:
                nc.vector.tensor_copy(o_sb, ps)
            eng = store_eng or (nc.sync if st % 2 == 0 else nc.gpsimd)
            eng.dma_start(
                out=out_d[b * S + st * 128: b * S + (st + 1) * 128, :], in_=o_sb)

        def alloc_attn(b):
            state[b, "ctxT"] = ctxp.tile([128, S], F32R, tag="ctxT", name="ctxT")
            state[b, "denom"] = dnp.tile([1, 2, S], F32, tag="denom", name="denom")

        def attention_qc(b, qc, inserts=()):
            QT, KT, V = state[b, "QT"], state[b, "KT"], state[b, "V"]
            ctxT, denom = state[b, "ctxT"], state[b, "denom"]
            sl = slice(qc * 1024, (qc + 1) * 1024)
            inserts = deque(inserts)
            pcs = [None, None]
            pss = [None, None]
            pending = deque()

            def scores(u, sk):
                pss[u] = ps_tile([128, 1024], "sA" if u == 0 else "sB")
                for hh in range(2):
                    nc.tensor.matmul(
                        pss[u][:, hh * 512:(hh + 1) * 512],
                        KT[u * DH:(u + 1) * DH, sk * 128:(sk + 1) * 128],
                        QT[u * DH:(u + 1) * DH,
                           qc * 1024 + hh * 512:qc * 1024 + (hh + 1) * 512],
                        start=True, stop=True)

            def expop(u, sk):
                e = expp.tile([128, 1024], F32R, tag="exp", name="exp_t")
                nc.scalar.activation(e, pss[u], Act.Exp, bias=zero_sb, scale=1.0)
                pending.append((u, sk, e))

            def ctx_drain(target_len):
                while len(pending) > target_len:
                    u, sk, e = pending.popleft()
                    if pcs[u] is None:
                        pcs[u] = ps_tile([DH + 1, 1024], "cA" if u == 0 else "cB")
                    for hh in range(2):
                        nc.tensor.matmul(
                            pcs[u][:, hh * 512:(hh + 1) * 512], V[:, sk, u, :],
                            e[:, hh * 512:(hh + 1) * 512],
                            start=(sk == 0), stop=(sk == NST - 1))

            scores(0, 0)
            scores(1, 0)
            for sk in range(NST):
                expop(0, sk)
                expop(1, sk)
                if sk + 1 < NST:
                    scores(0, sk + 1)
                if inserts:
                    inserts.popleft()()
                if sk + 1 < NST:
                    scores(1, sk + 1)
                if inserts:
                    ctx_drain(12)
                else:
                    ctx_drain(2)
            while inserts:
                inserts.popleft()()
            ctx_drain(0)

            for u in range(2):
                nc.vector.tensor_copy(ctxT[u * DH:(u + 1) * DH, sl], pcs[u][0:DH, :])
                nc.vector.tensor_copy(denom[0:1, u, sl], pcs[u][DH:DH + 1, :])

        def normalize(b, qc=None):
            ctxT, denom = state[b, "ctxT"], state[b, "denom"]
            sl = slice(0, S) if qc is None else slice(qc * 1024, (qc + 1) * 1024)
            nc.sync.dma_start(out=dn_d[:, sl], in_=denom[0:1, :, sl])
            key = (b, "rep")
            if key not in state:
                state[key] = dnp.tile([128, S], F32, tag="rep", name="rep")
            rep = state[key]
            for u in range(2):
                nc.sync.dma_start(out=rep[u * DH:(u + 1) * DH, sl],
                                  in_=_bcast_rows(dn_d[u:u + 1, sl], DH))
            nc.vector.reciprocal(rep[:, sl], rep[:, sl])
            nc.vector.tensor_mul(ctxT[:, sl], ctxT[:, sl], rep[:, sl].bitcast(F32R))

        def thunk(f, *a):
            def g():
                f(*a)
            return g

        # =========================== schedule ===========================
        load(0, (nc.sync, nc.scalar))
        alloc_proj(0)
        alloc_v(0)
        proj_chunk(0, "k", 0, "sA")
        proj_chunk(0, "q", 0, "sB")
        proj_chunk(0, "v", 0, "sA")
        for st in range(8):
            tr_one(0, st, "cA" if st % 2 == 0 else "cB")
        proj_chunk(0, "k", 1, "sB")
        proj_chunk(0, "q", 1, "sA")
        proj_chunk(0, "v", 1, "sB")
        for st in range(8, NST):
            tr_one(0, st, "cA" if st % 2 == 0 else "cB")
        load(1, (nc.sync, nc.gpsimd))  # overlaps attn0; ACT queue untouched

        alloc_attn(0)
        attention_qc(0, 0)
        normalize(0, 0)       # overlaps attn0-qc1
        attention_qc(0, 1)
        normalize(0, 1)

        alloc_proj(1)
        alloc_v(1)
        proj_chunk(1, "k", 0, "sA")
        proj_chunk(1, "q", 0, "sB")
        proj_chunk(1, "v", 0, "sA")
        for st in range(8):
            tr_one(1, st, "cA" if st % 2 == 0 else "cB")
        proj_chunk(1, "k", 1, "sB")
        proj_chunk(1, "q", 1, "sA")
        proj_chunk(1, "v", 1, "sB")
        for st in range(8, NST):
            tr_one(1, st, "cA" if st % 2 == 0 else "cB")
        for st in range(NST):
            outproj_st(0, st, ("sA", "sB", "cA", "cB")[st % 4], evac_act=(st % 2 == 1))

        alloc_attn(1)
        attention_qc(1, 0)
        normalize(1, 0)       # overlaps attn1-qc1 (DVE/DMA only, no PE)
        attention_qc(1, 1)
        normalize(1, 1)
        for st in range(NST):
            outproj_st(1, st, ("sA", "sB", "cA", "cB")[st % 4], evac_act=(st % 2 == 1),
                       store_eng=(nc.sync if st % 2 == 0 else nc.scalar))

    _split_sync_commands(nc)
    return nc


def _prepare(query, q_w, q_b, k_w, k_b, v_w, v_b, out_w):
    qt = np.ascontiguousarray(query.reshape(BS, D).T)  # [D, BS]
    in_maps = []
    for c in range(N_CORES):
        sl = slice(c * DPC, (c + 1) * DPC)
        in_maps.append({
            "qt": qt,
            "wq": np.ascontiguousarray(q_w[sl, :].T),
            "wk": np.ascontiguousarray(k_w[sl, :].T),
            "wv": np.ascontiguousarray(v_w[sl, :].T),
            "bq": np.ascontiguousarray(q_b[sl].reshape(DPC, 1)),
            "bk": np.ascontiguousarray(k_b[sl].reshape(DPC, 1)),
            "bv": np.ascontiguousarray(v_b[sl].reshape(DPC, 1)),
            "wo": np.ascontiguousarray(out_w[:, sl].T),
        })
    return in_maps


def kernel(query, mask, q_w, q_b, k_w, k_b, v_w, v_b, out_w, out_b):
    query = np.asarray(query, dtype=np.float32)
    q_w = np.asarray(q_w, dtype=np.float32); q_b = np.asarray(q_b, dtype=np.float32)
    k_w = np.asarray(k_w, dtype=np.float32); k_b = np.asarray(k_b, dtype=np.float32)
    v_w = np.asarray(v_w, dtype=np.float32); v_b = np.asarray(v_b, dtype=np.float32)
    out_w = np.asarray(out_w, dtype=np.float32); out_b = np.asarray(out_b, dtype=np.float32)

    in_maps = _prepare(query, q_w, q_b, k_w, k_b, v_w, v_b, out_w)
    nc = _build()
    res = run_bass_kernel_spmd(nc, in_maps, core_ids=list(range(N_CORES)))
    out = np.zeros((BS, D), dtype=np.float32)
    for c in range(N_CORES):
        out += res.results[c]["out_part"]
    out += out_b[None, :]
    return out.reshape(B, S, D)
